# revision 15
# baseline (speedup 1.0000x reference)
"""Multi-head attention (EMBED=384, 6 heads, S=1024, N=16) on 8 trn2 NeuronCores.

Strategy: data-parallel over batch (2 batches/core). Everything stays on-chip
per batch. Layouts chosen so no transposes are ever needed:
  - x[b] is [C, S] in DRAM == tokens^T, used directly as matmul rhs/lhsT.
  - qT/kT computed as [C_qk, S] (w_qk @ x) -> scoresT tiles = kT-tile.T @ qT,
    with the two heads of a pair packed into PE row groups (K=64 each).
  - v computed in [S, C_v] layout (x-tile.T @ w_vT) + a ones column -> attn@v
    yields outT [64, S] per head with the softmax row-sums in psum row 64.
  - exp on ACT with the 1/sqrt(d) scale folded in; no max-subtraction needed
    (|scores*scale| < ~6, exp is safe in fp32).
  - normalization: reciprocal of sums -> broadcast across 64 partitions via a
    tiny K=6 selector matmul -> elementwise multiply on catT; the output
    projection then emits finalT = [C, S], DMA'd straight to the output.
"""
import sys

sys.path.insert(0, "/opt/trn_rl_repo")
import numpy as np
import concourse.bass as bass
import concourse.tile as tile
from concourse import mybir
from concourse.bass import ts
from concourse.vector_clock import ScopedClock

f32 = mybir.dt.float32
f32r = mybir.dt.float32r

N, C, HW, S = 16, 384, 32, 1024
NH, HD = 6, 64
N3C = 3 * C  # 1152
N_CORES = 8
BPC = N // N_CORES  # batches per core
SCALE = HD**-0.5
MM_DT = f32r  # matmul dtype: f32r = full-rate tf32-like; f32 = 1/4-rate, precise

# ---------------------------------------------------------------------------
# Workarounds for walrus 1-sync-wait-per-instruction limit
# ---------------------------------------------------------------------------


def _patched_drain_and_barrier(self, tick_clock, wait_clock):
    nc = self.nc
    probe = nc.sync.nop(nofuse=True, hint="drain_waits")
    wait_clock.add_sem_waits(probe.ins, ScopedClock({None: tick_clock.global_clock}))
    inst = probe.ins
    si = inst.sync_info
    waits = list(si.on_wait) if si is not None else []
    if len(waits) > 1:
        inst.sync_info = mybir.SyncInfo(on_wait=[waits[0]], on_update=list(si.on_update))
        for w in waits[1:]:
            extra = nc.sync.nop(nofuse=True, hint="drain_waits")
            extra.ins.sync_info = mybir.SyncInfo(on_wait=[w], on_update=[])
    nc.sync.drain()
    nc.all_engine_barrier()
    assert self.sems is not None
    popped = nc._tile_sem_poison_stack.pop()
    assert popped is self._sem_poison
    nc.clear_and_free_semaphores(list(self.sems.allocated().values()))
    nc.all_engine_barrier()


tile.TileContext._drain_and_barrier = _patched_drain_and_barrier


def _split_multi_waits(nc):
    n_split = 0
    for fn in nc.m.functions:
        for bb in fn.blocks:
            insts = list(bb.instructions)
            out = []
            changed = False
            for inst in insts:
                si = getattr(inst, "sync_info", None)
                try:
                    waits = list(si.on_wait) if si is not None else []
                except Exception:
                    waits = []
                if len(waits) > 1:
                    for w in waits[:-1]:
                        nop = mybir.InstNoOp(name=f"waitsplit-{n_split}")
                        n_split += 1
                        nop.engine = inst.engine
                        nop.sync_info = mybir.SyncInfo(on_wait=[w], on_update=[])
                        out.append(nop)
                    inst.sync_info = mybir.SyncInfo(
                        on_wait=[waits[-1]], on_update=list(si.on_update)
                    )
                    changed = True
                out.append(inst)
            if changed:
                bb.instructions = out
    return n_split


# ---------------------------------------------------------------------------
# Kernel build
# ---------------------------------------------------------------------------


def _build(iters=1):
    nc = bass.Bass("TRN2", target_bir_lowering=False, debug=False, num_devices=N_CORES)
    xs = nc.declare_dram_parameter("xs", [BPC, C, S], MM_DT, isOutput=False)
    wqkvT_d = nc.declare_dram_parameter("wqkvT", [C, N3C], MM_DT, isOutput=False)
    woutT_d = nc.declare_dram_parameter("woutT", [C, C], MM_DT, isOutput=False)
    bout_d = nc.declare_dram_parameter("bout", [C], f32, isOutput=False)
    esel_d = nc.declare_dram_parameter("esel", [2, 128], MM_DT, isOutput=False)
    vones_d = nc.declare_dram_parameter("vones", [128, 8, NH], MM_DT, isOutput=False)
    out_d = nc.declare_dram_parameter("out", [BPC, C, S], f32, isOutput=True)

    with tile.TileContext(nc) as tc:
        with nc.allow_low_precision(reason="f32r (tf32-like) matmul pipeline"):
            _emit(nc, tc, xs, wqkvT_d, woutT_d, bout_d, esel_d, vones_d, out_d, iters)
    _split_multi_waits(nc)
    return nc


def _emit(nc, tc, xs, wqkvT_d, woutT_d, bout_d, esel_d, vones_d, out_d, iters=1):
    """Software-pipelined emission: the projection/norm work of neighboring
    batches is queued as filler closures and drained one per attention slot,
    so PE and ACT stay busy across batch boundaries. Matmuls that share a
    stationary operand are emitted adjacently (weight reuse is ~2x faster),
    and the two heads of a pair run concurrently in separate PE row groups.
    """
    import collections
    import contextlib

    ctx = contextlib.ExitStack()
    consts = ctx.enter_context(tc.tile_pool(name="consts", bufs=1))
    xpool = ctx.enter_context(tc.tile_pool(name="xpool", bufs=2))
    qkpool = ctx.enter_context(tc.tile_pool(name="qkpool", bufs=2))
    vpool = ctx.enter_context(tc.tile_pool(name="vpool", bufs=2))
    attnpool = ctx.enter_context(tc.tile_pool(name="attnpool", bufs=6))
    catpool = ctx.enter_context(tc.tile_pool(name="catpool", bufs=2))
    rpool = ctx.enter_context(tc.tile_pool(name="rpool", bufs=4))
    rtpool = ctx.enter_context(tc.tile_pool(name="rtpool", bufs=2))
    fpool = ctx.enter_context(tc.tile_pool(name="fpool", bufs=4))
    ps_wide = ctx.enter_context(tc.tile_pool(name="ps_wide", bufs=2, space="PSUM"))
    ps_o = ctx.enter_context(tc.tile_pool(name="ps_o", bufs=4, space="PSUM"))
    ps_small = ctx.enter_context(tc.tile_pool(name="ps_small", bufs=2, space="PSUM"))

    # ---- constants (loaded once) ----
    wq = consts.tile([128, 3, N3C], MM_DT)  # w_qkv^T   k-tile-major
    for k, eng in zip(range(3), (nc.sync, nc.scalar, nc.gpsimd)):
        eng.dma_start(out=wq[:, k, :], in_=wqkvT_d[ts(k, 128), :])
    wo = consts.tile([128, 3, C], MM_DT)  # w_out^T
    bo = consts.tile([128, 3], f32)
    esel = consts.tile([2, 128], MM_DT)  # rows select lower/upper head of a pair

    def _load_late_consts():
        for k in range(3):
            nc.gpsimd.dma_start(out=wo[:, k, :], in_=woutT_d[ts(k, 128), :])
            nc.gpsimd.dma_start(
                out=bo[:, k : k + 1],
                in_=bout_d[ts(k, 128)].rearrange("(p o) -> p o", o=1),
            )
        nc.gpsimd.dma_start(out=esel, in_=esel_d[:, :])

    mm = nc.tensor.matmul
    EXP = mybir.ActivationFunctionType.Exp

    nseq = iters * BPC
    state = {}  # seq -> dict(x, qkT, v, cat, recip)
    filler = collections.deque()
    pending_pro = collections.Counter()  # seq -> un-run prologue closures

    def drain(k=1):
        for _ in range(k):
            if filler:
                filler.popleft()()

    def drain_prologue(seq):
        while pending_pro[seq] > 0:
            filler.popleft()()

    def queue_prologue(seq):
        b = seq % BPC
        st = state.setdefault(seq, {})

        def xload():
            x_sb = xpool.tile([128, 3, S], MM_DT, tag="x", name=f"x_{seq}")
            for k, eng in zip(range(3), (nc.sync, nc.scalar, nc.gpsimd)):
                eng.dma_start(out=x_sb[:, k, :], in_=xs[b, ts(k, 128), :])
            st["x"] = x_sb
            st["qkT"] = qkpool.tile([128, 6, S], MM_DT, tag="qkT", name=f"qkT_{seq}")
            v_sb = vpool.tile([128, 8, NH, HD + 1], MM_DT, tag="v", name=f"v_{seq}")
            nc.gpsimd.dma_start(
                out=v_sb[:, :, :, HD : HD + 1],
                in_=vones_d[:, :, :].rearrange("p a (h o) -> p a h o", o=1),
            )
            st["v"] = v_sb
            pending_pro[seq] -= 1

        filler.append(xload)

        def qk_group(j):
            def f():
                x_sb, qkT = st["x"], st["qkT"]
                pq = [
                    ps_small.tile([128, 512], f32, tag="small", name=f"pq_{seq}_{j}_{u}")
                    for u in range(2)
                ]
                for k in range(3):
                    for u in range(2):
                        mm(pq[u], wq[:, k, ts(j, 128)], x_sb[:, k, ts(u, 512)],
                           start=(k == 0), stop=(k == 2))
                for u in range(2):
                    nc.vector.tensor_copy(out=qkT[:, j, ts(u, 512)], in_=pq[u])
                pending_pro[seq] -= 1

            return f

        for j in range(6):
            filler.append(qk_group(j))

        def v_group(i):
            def f():
                x_sb, v_sb = st["x"], st["v"]
                pv = ps_small.tile([128, C], f32, tag="small", name=f"pv_{seq}_{i}")
                for k in range(3):
                    mm(pv, x_sb[:, k, ts(i, 128)], wq[:, k, 2 * C : N3C],
                       start=(k == 0), stop=(k == 2))
                nc.vector.tensor_copy(
                    out=v_sb[:, i, :, 0:HD],
                    in_=pv.rearrange("p (h d) -> p h d", h=NH),
                )
                pending_pro[seq] -= 1

            return f

        for i in range(8):
            filler.append(v_group(i))
        pending_pro[seq] = 15

    def queue_norm(seq, g):
        st = state[seq]
        recip_g = st["recip"][g]

        def f():
            catT = st["cat"]
            for u in range(2):
                pr = ps_small.tile([128, 512], f32, tag="small", name=f"pr_{seq}_{g}_{u}")
                mm(pr, esel, recip_g[:, ts(u, 512)], start=True, stop=True)
                nc.vector.tensor_mul(
                    out=catT[:, g, ts(u, 512)], in0=catT[:, g, ts(u, 512)], in1=pr
                )

        filler.append(f)

    def queue_epilogue(seq):
        b = seq % BPC
        st = state[seq]

        def proj_unit(j):
            def f():
                catT = st["cat"]
                pf = [
                    ps_small.tile([128, 512], f32, tag="small", name=f"pf_{seq}_{j}_{u}")
                    for u in range(2)
                ]
                for k in range(3):
                    for u in range(2):
                        mm(pf[u], wo[:, k, ts(j, 128)], catT[:, k, ts(u, 512)],
                           start=(k == 0), stop=(k == 2))
                for u in range(2):
                    fin = fpool.tile([128, 512], f32, tag="fin", name=f"fin_{seq}_{j}_{u}")
                    nc.vector.tensor_scalar_add(out=fin, in0=pf[u], scalar1=bo[:, j : j + 1])
                    nc.sync.dma_start(out=out_d[b, ts(j, 128), ts(u, 512)], in_=fin)
                # free catT/recip state once the last projection is emitted
                if j == 2:
                    state.pop(seq, None)

            return f

        for j in range(3):
            filler.append(proj_unit(j))

    def emit_pairs(seq):
        st = state[seq]
        qkT, v_sb = st["qkT"], st["v"]
        catT = catpool.tile([128, 3, S], MM_DT, tag="cat", name=f"cat_{seq}")
        st["cat"] = catT
        st["recip"] = {}

        for g in range(3):
            hA, hB = 2 * g, 2 * g + 1
            psoA = [
                ps_o.tile([HD + 1, 512], f32, tag="o", name=f"psoA_{seq}_{g}_{u}")
                for u in range(2)
            ]
            psoB = [
                ps_o.tile([HD + 1, 512], f32, tag="o", name=f"psoB_{seq}_{g}_{u}")
                for u in range(2)
            ]
            prev = None  # (aA, aB, t) awaiting attnv

            def attnv(aA, aB, t):
                for u in range(2):
                    mm(psoA[u], v_sb[:, t, hA, :], aA[:, ts(u, 512)],
                       start=(t == 0), stop=(t == 7))
                for u in range(2):
                    mm(psoB[u], v_sb[:, t, hB, :], aB[:, ts(u, 512)],
                       start=(t == 0), stop=(t == 7))

            for t in range(8):
                aA = attnpool.tile([128, S], MM_DT, tag="attn", name=f"aA_{seq}_{g}_{t}")
                aB = attnpool.tile([128, S], MM_DT, tag="attn", name=f"aB_{seq}_{g}_{t}")
                pwA = [
                    ps_wide.tile([128, 512], f32, tag="wide", name=f"pwA_{seq}_{g}_{t}_{u}")
                    for u in range(2)
                ]
                mm(pwA[0], qkT[0:64, 3 + g, ts(t, 128)], qkT[0:64, g, 0:512],
                   start=True, stop=True, tile_position=(0, 0))
                mm(pwA[1], qkT[0:64, 3 + g, ts(t, 128)], qkT[0:64, g, 512:1024],
                   start=True, stop=True, tile_position=(0, 0))
                nc.scalar.activation(out=aA[:, 0:512], in_=pwA[0], func=EXP, scale=SCALE)
                pwB0 = ps_wide.tile([128, 512], f32, tag="wide", name=f"pwB_{seq}_{g}_{t}_0")
                mm(pwB0, qkT[64:128, 3 + g, ts(t, 128)], qkT[64:128, g, 0:512],
                   start=True, stop=True, tile_position=(64, 0))
                nc.scalar.activation(out=aA[:, 512:1024], in_=pwA[1], func=EXP, scale=SCALE)
                pwB1 = ps_wide.tile([128, 512], f32, tag="wide", name=f"pwB_{seq}_{g}_{t}_1")
                mm(pwB1, qkT[64:128, 3 + g, ts(t, 128)], qkT[64:128, g, 512:1024],
                   start=True, stop=True, tile_position=(64, 0))
                if prev is not None:
                    attnv(*prev)
                nc.scalar.activation(out=aB[:, 0:512], in_=pwB0, func=EXP, scale=SCALE)
                nc.scalar.activation(out=aB[:, 512:1024], in_=pwB1, func=EXP, scale=SCALE)
                prev = (aA, aB, t)
                if t < 7:
                    drain(1)
            attnv(*prev)

            recip_g = rpool.tile([2, S], MM_DT, tag="recip", name=f"recip_{seq}_{g}")
            st["recip"][g] = recip_g
            for h, pso2 in ((hA, psoA), (hB, psoB)):
                po = (h % 2) * 64
                rt = rtpool.tile([1, S], MM_DT, tag="rt", name=f"rt_{seq}_{h}")
                for u in range(2):
                    nc.vector.reciprocal(
                        out=rt[0:1, ts(u, 512)], in_=pso2[u][HD : HD + 1, :]
                    )
                    nc.scalar.copy(
                        out=catT[po : po + HD, h // 2, ts(u, 512)], in_=pso2[u][0:HD, :]
                    )
                nc.gpsimd.dma_start(out=recip_g[h % 2 : h % 2 + 1, :], in_=rt)
            queue_norm(seq, g)
            drain(1)

    # ---------------- the pipeline ----------------
    queue_prologue(0)
    filler.append(_load_late_consts)
    for seq in range(nseq):
        drain_prologue(seq)
        if seq + 1 < nseq:
            queue_prologue(seq + 1)
        emit_pairs(seq)
        queue_epilogue(seq)
    drain(len(filler))

    ctx.close()


_CACHED = None


def _get_nc():
    global _CACHED
    if _CACHED is None:
        _CACHED = _build()
    return _CACHED


def _esel_np():
    e = np.zeros((2, 128), np.float32)
    e[0, 0:64] = 1.0
    e[1, 64:128] = 1.0
    return e


def _in_maps(x, w_qkv, w_out, b_out):
    x = np.ascontiguousarray(np.asarray(x, dtype=np.float32))
    xs_full = x.reshape(N, C, S)
    wqkvT = np.ascontiguousarray(np.asarray(w_qkv, np.float32).T)
    woutT = np.ascontiguousarray(np.asarray(w_out, np.float32).T)
    bout = np.ascontiguousarray(np.asarray(b_out, np.float32))
    esel = _esel_np()
    return [
        {
            "xs": xs_full[i * BPC : (i + 1) * BPC],
            "wqkvT": wqkvT,
            "woutT": woutT,
            "bout": bout,
            "esel": esel,
            "vones": np.ones((128, 8, NH), np.float32),
        }
        for i in range(N_CORES)
    ]


def kernel(x, w_qkv, w_out, b_out):
    from concourse.bass_utils import run_bass_kernel_spmd

    nc = _get_nc()
    res = run_bass_kernel_spmd(nc, _in_maps(x, w_qkv, w_out, b_out), list(range(N_CORES)))
    out = np.concatenate([res.results[i]["out"] for i in range(N_CORES)], axis=0)
    return out.reshape(N, C, HW, HW)


# revision 16
# speedup vs baseline: 1.0093x; 1.0093x over previous
"""Multi-head attention (EMBED=384, 6 heads, S=1024, N=16) on 8 trn2 NeuronCores.

Strategy: data-parallel over batch (2 batches/core). Everything stays on-chip
per batch. Layouts chosen so no transposes are ever needed:
  - x[b] is [C, S] in DRAM == tokens^T, used directly as matmul rhs/lhsT.
  - qT/kT computed as [C_qk, S] (w_qk @ x) -> scoresT tiles = kT-tile.T @ qT,
    with the two heads of a pair packed into PE row groups (K=64 each).
  - v computed in [S, C_v] layout (x-tile.T @ w_vT) + a ones column -> attn@v
    yields outT [64, S] per head with the softmax row-sums in psum row 64.
  - exp on ACT with the 1/sqrt(d) scale folded in; no max-subtraction needed
    (|scores*scale| < ~6, exp is safe in fp32).
  - normalization: reciprocal of sums -> broadcast across 64 partitions via a
    tiny K=6 selector matmul -> elementwise multiply on catT; the output
    projection then emits finalT = [C, S], DMA'd straight to the output.
"""
import sys

sys.path.insert(0, "/opt/trn_rl_repo")
import numpy as np
import concourse.bass as bass
import concourse.tile as tile
from concourse import mybir
from concourse.bass import ts
from concourse.vector_clock import ScopedClock

f32 = mybir.dt.float32
f32r = mybir.dt.float32r

N, C, HW, S = 16, 384, 32, 1024
NH, HD = 6, 64
N3C = 3 * C  # 1152
N_CORES = 8
BPC = N // N_CORES  # batches per core
SCALE = HD**-0.5
MM_DT = f32r  # matmul dtype: f32r = full-rate tf32-like; f32 = 1/4-rate, precise

# ---------------------------------------------------------------------------
# Workarounds for walrus 1-sync-wait-per-instruction limit
# ---------------------------------------------------------------------------


def _patched_drain_and_barrier(self, tick_clock, wait_clock):
    nc = self.nc
    probe = nc.sync.nop(nofuse=True, hint="drain_waits")
    wait_clock.add_sem_waits(probe.ins, ScopedClock({None: tick_clock.global_clock}))
    inst = probe.ins
    si = inst.sync_info
    waits = list(si.on_wait) if si is not None else []
    if len(waits) > 1:
        inst.sync_info = mybir.SyncInfo(on_wait=[waits[0]], on_update=list(si.on_update))
        for w in waits[1:]:
            extra = nc.sync.nop(nofuse=True, hint="drain_waits")
            extra.ins.sync_info = mybir.SyncInfo(on_wait=[w], on_update=[])
    nc.sync.drain()
    nc.all_engine_barrier()
    assert self.sems is not None
    popped = nc._tile_sem_poison_stack.pop()
    assert popped is self._sem_poison
    nc.clear_and_free_semaphores(list(self.sems.allocated().values()))
    nc.all_engine_barrier()


tile.TileContext._drain_and_barrier = _patched_drain_and_barrier


def _split_multi_waits(nc):
    n_split = 0
    for fn in nc.m.functions:
        for bb in fn.blocks:
            insts = list(bb.instructions)
            out = []
            changed = False
            for inst in insts:
                si = getattr(inst, "sync_info", None)
                try:
                    waits = list(si.on_wait) if si is not None else []
                except Exception:
                    waits = []
                if len(waits) > 1:
                    for w in waits[:-1]:
                        nop = mybir.InstNoOp(name=f"waitsplit-{n_split}")
                        n_split += 1
                        nop.engine = inst.engine
                        nop.sync_info = mybir.SyncInfo(on_wait=[w], on_update=[])
                        out.append(nop)
                    inst.sync_info = mybir.SyncInfo(
                        on_wait=[waits[-1]], on_update=list(si.on_update)
                    )
                    changed = True
                out.append(inst)
            if changed:
                bb.instructions = out
    return n_split


# ---------------------------------------------------------------------------
# Kernel build
# ---------------------------------------------------------------------------


def _build(iters=1):
    nc = bass.Bass("TRN2", target_bir_lowering=False, debug=False, num_devices=N_CORES)
    xs = nc.declare_dram_parameter("xs", [BPC, C, S], MM_DT, isOutput=False)
    wqkvT_d = nc.declare_dram_parameter("wqkvT", [C, N3C], MM_DT, isOutput=False)
    woutT_d = nc.declare_dram_parameter("woutT", [C, C], MM_DT, isOutput=False)
    bout_d = nc.declare_dram_parameter("bout", [C], f32, isOutput=False)
    esel_d = nc.declare_dram_parameter("esel", [2, 128], MM_DT, isOutput=False)
    vones_d = nc.declare_dram_parameter("vones", [128, 8, NH], MM_DT, isOutput=False)
    out_d = nc.declare_dram_parameter("out", [BPC, C, S], f32, isOutput=True)

    with tile.TileContext(nc) as tc:
        with nc.allow_low_precision(reason="f32r (tf32-like) matmul pipeline"):
            _emit(nc, tc, xs, wqkvT_d, woutT_d, bout_d, esel_d, vones_d, out_d, iters)
    _split_multi_waits(nc)
    return nc


def _emit(nc, tc, xs, wqkvT_d, woutT_d, bout_d, esel_d, vones_d, out_d, iters=1):
    """Software-pipelined emission: the projection/norm work of neighboring
    batches is queued as filler closures and drained one per attention slot,
    so PE and ACT stay busy across batch boundaries. Matmuls that share a
    stationary operand are emitted adjacently (weight reuse is ~2x faster),
    and the two heads of a pair run concurrently in separate PE row groups.
    """
    import collections
    import contextlib

    ctx = contextlib.ExitStack()
    consts = ctx.enter_context(tc.tile_pool(name="consts", bufs=1))
    xpool = ctx.enter_context(tc.tile_pool(name="xpool", bufs=2))
    qkpool = ctx.enter_context(tc.tile_pool(name="qkpool", bufs=2))
    vpool = ctx.enter_context(tc.tile_pool(name="vpool", bufs=2))
    attnpool = ctx.enter_context(tc.tile_pool(name="attnpool", bufs=6))
    catpool = ctx.enter_context(tc.tile_pool(name="catpool", bufs=2))
    rpool = ctx.enter_context(tc.tile_pool(name="rpool", bufs=4))
    rtpool = ctx.enter_context(tc.tile_pool(name="rtpool", bufs=2))
    fpool = ctx.enter_context(tc.tile_pool(name="fpool", bufs=4))
    ps_wide = ctx.enter_context(tc.tile_pool(name="ps_wide", bufs=2, space="PSUM"))
    ps_o = ctx.enter_context(tc.tile_pool(name="ps_o", bufs=4, space="PSUM"))
    ps_small = ctx.enter_context(tc.tile_pool(name="ps_small", bufs=2, space="PSUM"))

    # ---- constants (loaded once) ----
    wq = consts.tile([128, 3, N3C], MM_DT)  # w_qkv^T   k-tile-major
    for k, eng in zip(range(3), (nc.sync, nc.gpsimd, nc.sync)):
        eng.dma_start(out=wq[:, k, :], in_=wqkvT_d[ts(k, 128), :])
    wo = consts.tile([128, 3, C], MM_DT)  # w_out^T
    bo = consts.tile([128, 3], f32)
    esel = consts.tile([2, 128], MM_DT)  # rows select lower/upper head of a pair

    def _load_late_consts():
        for k in range(3):
            nc.gpsimd.dma_start(out=wo[:, k, :], in_=woutT_d[ts(k, 128), :])
            nc.gpsimd.dma_start(
                out=bo[:, k : k + 1],
                in_=bout_d[ts(k, 128)].rearrange("(p o) -> p o", o=1),
            )
        nc.gpsimd.dma_start(out=esel, in_=esel_d[:, :])

    mm = nc.tensor.matmul
    EXP = mybir.ActivationFunctionType.Exp

    nseq = iters * BPC
    state = {}  # seq -> dict(x, qkT, v, cat, recip)
    filler = collections.deque()
    pending_pro = collections.Counter()  # seq -> un-run prologue closures

    def drain(k=1):
        for _ in range(k):
            if filler:
                filler.popleft()()

    def drain_prologue(seq):
        while pending_pro[seq] > 0:
            filler.popleft()()

    def queue_prologue(seq):
        b = seq % BPC
        st = state.setdefault(seq, {})

        def xload():
            x_sb = xpool.tile([128, 3, S], MM_DT, tag="x", name=f"x_{seq}")
            for k, eng in zip(range(3), (nc.scalar, nc.gpsimd, nc.scalar)):
                eng.dma_start(out=x_sb[:, k, :], in_=xs[b, ts(k, 128), :])
            st["x"] = x_sb
            st["qkT"] = qkpool.tile([128, 6, S], MM_DT, tag="qkT", name=f"qkT_{seq}")
            v_sb = vpool.tile([128, 8, NH, HD + 1], MM_DT, tag="v", name=f"v_{seq}")
            nc.gpsimd.dma_start(
                out=v_sb[:, :, :, HD : HD + 1],
                in_=vones_d[:, :, :].rearrange("p a (h o) -> p a h o", o=1),
            )
            st["v"] = v_sb
            pending_pro[seq] -= 1

        filler.append(xload)

        def qk_group(j):
            def f():
                x_sb, qkT = st["x"], st["qkT"]
                pq = [
                    ps_small.tile([128, 512], f32, tag="small", name=f"pq_{seq}_{j}_{u}")
                    for u in range(2)
                ]
                for k in range(3):
                    for u in range(2):
                        mm(pq[u], wq[:, k, ts(j, 128)], x_sb[:, k, ts(u, 512)],
                           start=(k == 0), stop=(k == 2))
                for u in range(2):
                    nc.vector.tensor_copy(out=qkT[:, j, ts(u, 512)], in_=pq[u])
                pending_pro[seq] -= 1

            return f

        for j in range(6):
            filler.append(qk_group(j))

        def v_group(i):
            def f():
                x_sb, v_sb = st["x"], st["v"]
                pv = ps_small.tile([128, C], f32, tag="small", name=f"pv_{seq}_{i}")
                for k in range(3):
                    mm(pv, x_sb[:, k, ts(i, 128)], wq[:, k, 2 * C : N3C],
                       start=(k == 0), stop=(k == 2))
                nc.vector.tensor_copy(
                    out=v_sb[:, i, :, 0:HD],
                    in_=pv.rearrange("p (h d) -> p h d", h=NH),
                )
                pending_pro[seq] -= 1

            return f

        for i in range(8):
            filler.append(v_group(i))
        pending_pro[seq] = 15

    def queue_norm(seq, g):
        st = state[seq]
        recip_g = st["recip"][g]

        def f():
            catT = st["cat"]
            for u in range(2):
                pr = ps_small.tile([128, 512], f32, tag="small", name=f"pr_{seq}_{g}_{u}")
                mm(pr, esel, recip_g[:, ts(u, 512)], start=True, stop=True)
                nc.vector.tensor_mul(
                    out=catT[:, g, ts(u, 512)], in0=catT[:, g, ts(u, 512)], in1=pr
                )

        filler.append(f)

    def queue_epilogue(seq):
        b = seq % BPC
        st = state[seq]

        def proj_unit(j):
            def f():
                catT = st["cat"]
                pf = [
                    ps_small.tile([128, 512], f32, tag="small", name=f"pf_{seq}_{j}_{u}")
                    for u in range(2)
                ]
                for k in range(3):
                    for u in range(2):
                        mm(pf[u], wo[:, k, ts(j, 128)], catT[:, k, ts(u, 512)],
                           start=(k == 0), stop=(k == 2))
                for u in range(2):
                    fin = fpool.tile([128, 512], f32, tag="fin", name=f"fin_{seq}_{j}_{u}")
                    nc.vector.tensor_scalar_add(out=fin, in0=pf[u], scalar1=bo[:, j : j + 1])
                    nc.sync.dma_start(out=out_d[b, ts(j, 128), ts(u, 512)], in_=fin)
                # free catT/recip state once the last projection is emitted
                if j == 2:
                    state.pop(seq, None)

            return f

        for j in range(3):
            filler.append(proj_unit(j))

    def emit_pairs(seq):
        st = state[seq]
        qkT, v_sb = st["qkT"], st["v"]
        catT = catpool.tile([128, 3, S], MM_DT, tag="cat", name=f"cat_{seq}")
        st["cat"] = catT
        st["recip"] = {}

        for g in range(3):
            hA, hB = 2 * g, 2 * g + 1
            psoA = [
                ps_o.tile([HD + 1, 512], f32, tag="o", name=f"psoA_{seq}_{g}_{u}")
                for u in range(2)
            ]
            psoB = [
                ps_o.tile([HD + 1, 512], f32, tag="o", name=f"psoB_{seq}_{g}_{u}")
                for u in range(2)
            ]
            prev = None  # (aA, aB, t) awaiting attnv

            def attnv(aA, aB, t):
                for u in range(2):
                    mm(psoA[u], v_sb[:, t, hA, :], aA[:, ts(u, 512)],
                       start=(t == 0), stop=(t == 7))
                for u in range(2):
                    mm(psoB[u], v_sb[:, t, hB, :], aB[:, ts(u, 512)],
                       start=(t == 0), stop=(t == 7))

            for t in range(8):
                aA = attnpool.tile([128, S], MM_DT, tag="attn", name=f"aA_{seq}_{g}_{t}")
                aB = attnpool.tile([128, S], MM_DT, tag="attn", name=f"aB_{seq}_{g}_{t}")
                pwA = [
                    ps_wide.tile([128, 512], f32, tag="wide", name=f"pwA_{seq}_{g}_{t}_{u}")
                    for u in range(2)
                ]
                mm(pwA[0], qkT[0:64, 3 + g, ts(t, 128)], qkT[0:64, g, 0:512],
                   start=True, stop=True, tile_position=(0, 0))
                mm(pwA[1], qkT[0:64, 3 + g, ts(t, 128)], qkT[0:64, g, 512:1024],
                   start=True, stop=True, tile_position=(0, 0))
                nc.scalar.activation(out=aA[:, 0:512], in_=pwA[0], func=EXP, scale=SCALE)
                pwB0 = ps_wide.tile([128, 512], f32, tag="wide", name=f"pwB_{seq}_{g}_{t}_0")
                mm(pwB0, qkT[64:128, 3 + g, ts(t, 128)], qkT[64:128, g, 0:512],
                   start=True, stop=True, tile_position=(64, 0))
                nc.scalar.activation(out=aA[:, 512:1024], in_=pwA[1], func=EXP, scale=SCALE)
                pwB1 = ps_wide.tile([128, 512], f32, tag="wide", name=f"pwB_{seq}_{g}_{t}_1")
                mm(pwB1, qkT[64:128, 3 + g, ts(t, 128)], qkT[64:128, g, 512:1024],
                   start=True, stop=True, tile_position=(64, 0))
                if prev is not None:
                    attnv(*prev)
                nc.scalar.activation(out=aB[:, 0:512], in_=pwB0, func=EXP, scale=SCALE)
                nc.scalar.activation(out=aB[:, 512:1024], in_=pwB1, func=EXP, scale=SCALE)
                prev = (aA, aB, t)
                if t < 7:
                    drain(1)
            attnv(*prev)

            recip_g = rpool.tile([2, S], MM_DT, tag="recip", name=f"recip_{seq}_{g}")
            st["recip"][g] = recip_g
            for h, pso2 in ((hA, psoA), (hB, psoB)):
                po = (h % 2) * 64
                rt = rtpool.tile([1, S], MM_DT, tag="rt", name=f"rt_{seq}_{h}")
                for u in range(2):
                    nc.vector.reciprocal(
                        out=rt[0:1, ts(u, 512)], in_=pso2[u][HD : HD + 1, :]
                    )
                    nc.scalar.copy(
                        out=catT[po : po + HD, h // 2, ts(u, 512)], in_=pso2[u][0:HD, :]
                    )
                nc.gpsimd.dma_start(out=recip_g[h % 2 : h % 2 + 1, :], in_=rt)
            queue_norm(seq, g)
            drain(1)

    # ---------------- the pipeline ----------------
    queue_prologue(0)
    filler.append(_load_late_consts)
    for seq in range(nseq):
        drain_prologue(seq)
        if seq + 1 < nseq:
            queue_prologue(seq + 1)
        emit_pairs(seq)
        queue_epilogue(seq)
    drain(len(filler))

    ctx.close()


_CACHED = None


def _get_nc():
    global _CACHED
    if _CACHED is None:
        _CACHED = _build()
    return _CACHED


def _esel_np():
    e = np.zeros((2, 128), np.float32)
    e[0, 0:64] = 1.0
    e[1, 64:128] = 1.0
    return e


def _in_maps(x, w_qkv, w_out, b_out):
    x = np.ascontiguousarray(np.asarray(x, dtype=np.float32))
    xs_full = x.reshape(N, C, S)
    wqkvT = np.ascontiguousarray(np.asarray(w_qkv, np.float32).T)
    woutT = np.ascontiguousarray(np.asarray(w_out, np.float32).T)
    bout = np.ascontiguousarray(np.asarray(b_out, np.float32))
    esel = _esel_np()
    return [
        {
            "xs": xs_full[i * BPC : (i + 1) * BPC],
            "wqkvT": wqkvT,
            "woutT": woutT,
            "bout": bout,
            "esel": esel,
            "vones": np.ones((128, 8, NH), np.float32),
        }
        for i in range(N_CORES)
    ]


def kernel(x, w_qkv, w_out, b_out):
    from concourse.bass_utils import run_bass_kernel_spmd

    nc = _get_nc()
    res = run_bass_kernel_spmd(nc, _in_maps(x, w_qkv, w_out, b_out), list(range(N_CORES)))
    out = np.concatenate([res.results[i]["out"] for i in range(N_CORES)], axis=0)
    return out.reshape(N, C, HW, HW)
